# revision 22
# baseline (speedup 1.0000x reference)
"""GQA FlashAttention (RMSNorm QK + RoPE, causal) on 8 TRN2 NeuronCores.

Sharding: tensor-parallel over heads (core c owns q-heads 4c..4c+3 and
kv-head c; the GQA group is fully local). Head-pair-chunked bf16
AllToAlls re-shard the attention output from head-parallel to
row-parallel; each core then computes its 256 output rows against the
full (SBUF-resident) Wo.

v3 structure:
- x is SBUF-resident per 512-column block ([128,16,512] bf16, 2-deep):
  each block is loaded ONCE (8MB HBM traffic instead of 16MB) and both
  projection passes read SBUF, so the PE k-loops carry no DMA deps.
- x block j+1's load descriptors are GATED behind block j's q-norm via
  a tiny qt->DRAM store that head-blocks the sync queue: the 16 SDMA
  engines round-robin over every queued DMA at packet granularity, so
  without the gate the block-0-critical loads finish only when ALL
  queued head traffic finishes (~45us measured).
- Attention runs PAIR-MAJOR: all four blocks' head-pair 0 first, so
  the pair-0 AllToAll (532KB, ~20-30us) fires ~45us earlier and
  transfers entirely under pair-1's attention; the pair-1 AllToAll is
  covered by the even-half of the out-projection.
- Diagonal causal mask: 0/1 bf16 multiply on the exp output (SBUF,
  2x DVE mode) instead of -1e9 f32 adds on PSUM scores.
- Norm chain evictions in bf16 (squares still computed from f32 PSUM);
  output stored bf16 and upcast on the host.
- bf16 operands everywhere on the PE, fp32 accumulation in PSUM;
  fused emission keeps the in-order PE stream dense (HAM stays warm);
  row-packed score matmuls (heads 2p/2p+1 in PE row groups 0/64);
  batched softmax ([128,2,512] exp ACTIVATEs, denominators ride a
  ones-column in V, normalization folded to reciprocal+broadcast on
  the re-sharded output); rsqrt = exp(-0.5*ln(var+eps)) so the whole
  kernel needs ONE activation table load.
"""

import sys

sys.path.insert(0, "/opt/trn_rl_repo")

import numpy as np
import ml_dtypes
import concourse.bass as bass  # noqa: F401
import concourse.tile as tile
from concourse import mybir, bacc
import concourse.bacc as bacc_mod
from concourse.bass_utils import run_bass_kernel_spmd
from concourse.hw_specs import get_activation_tables as _orig_get_tables
from concourse.masks import make_identity

N_CORES = 8
D_IN = 2048
SEQ = 2048
N_HEADS = 32
N_KV = 8
HD = 64
HPC = N_HEADS // N_CORES  # 4 q heads per core
EPS = 1e-6

F32 = mybir.dt.float32
F32R = mybir.dt.float32r
BF16 = mybir.dt.bfloat16
BF16_NP = ml_dtypes.bfloat16

KT_TILES = D_IN // 128
QB = 512
NQB = SEQ // QB  # 4
ROWS_PER_CORE = SEQ // N_CORES  # 256
AF = mybir.ActivationFunctionType

_ONE_TABLE = "natural_log_exp_and_others"


def _pinned_tables(arch):
    tabs = _orig_get_tables(arch)
    return {n: (fs if n == _ONE_TABLE else set()) for n, fs in tabs.items()}


def _build():
    bacc_mod.get_activation_tables = _pinned_tables
    nc = bacc.Bacc(num_devices=N_CORES)

    xT = nc.dram_tensor("xT", [128, NQB, KT_TILES, QB], BF16, kind="ExternalInput")
    wq = nc.dram_tensor("wq", [128, KT_TILES, HPC * HD], BF16, kind="ExternalInput")
    wkv = nc.dram_tensor("wkv", [128, KT_TILES, 2 * HD], BF16, kind="ExternalInput")
    wo = nc.dram_tensor("wo", [128, KT_TILES, D_IN], BF16, kind="ExternalInput")
    coswq = nc.dram_tensor("coswq", [128, SEQ], BF16, kind="ExternalInput")
    sinwq = nc.dram_tensor("sinwq", [128, SEQ], BF16, kind="ExternalInput")
    coswk = nc.dram_tensor("coswk", [64, SEQ], BF16, kind="ExternalInput")
    sinwk = nc.dram_tensor("sinwk", [64, SEQ], BF16, kind="ExternalInput")
    sel16 = nc.dram_tensor("sel16", [2 * N_CORES, 2 * N_CORES, 128], F32R, kind="ExternalInput")

    out = nc.dram_tensor("out", [ROWS_PER_CORE, D_IN], BF16, kind="ExternalOutput")

    with tile.TileContext(nc) as tc:
        with (
            tc.tile_pool(name="persist", bufs=1) as pers,
            tc.tile_pool(name="dram", bufs=1, space="DRAM") as dram,
        ):
            # ---- persistent SBUF preloads (contiguous, host-transposed) ----
            # weights preload from the (otherwise idle) scalar queue so the
            # sync queue starts issuing x chunks immediately
            wq_sb = pers.tile([128, KT_TILES, HPC * HD], BF16)
            wkv_sb = pers.tile([128, KT_TILES, 2 * HD], BF16)
            nc.scalar.dma_start(wq_sb[:, 0:8, :], wq[:, 0:8, :])
            nc.scalar.dma_start(wkv_sb[:], wkv[:])
            nc.scalar.dma_start(wq_sb[:, 8:16, :], wq[:, 8:16, :])

            cq_sb = pers.tile([128, SEQ], BF16)
            sq_sb = pers.tile([128, SEQ], BF16)
            ck_sb = pers.tile([64, SEQ], BF16)
            sk_sb = pers.tile([64, SEQ], BF16)
            mask_sb = pers.tile([128, 2, 128], BF16)

            def load_rope_tables():
                # issued on the sync queue AFTER block-0's x chunks: in-order
                # issue keeps the first matmuls' data ahead of these
                nc.sync.dma_start(ck_sb[:], coswk[:])
                nc.sync.dma_start(sk_sb[:], sinwk[:])
                nc.sync.dma_start(cq_sb[:], coswq[:])
                nc.sync.dma_start(sq_sb[:], sinwq[:])

            wo_sb = pers.tile([128, KT_TILES, D_IN], BF16)  # 8 MB
            sel_sb = pers.tile([2 * N_CORES, 2 * N_CORES, 128], F32R)

            ident = pers.tile([128, 128], F32)
            make_identity(nc, ident[:])
            eps_sb = pers.tile([128, 1], F32)
            nc.vector.memset(eps_sb[:], EPS)
            # block-diagonal ones (64x64 blocks), bf16, built on-chip: sums
            # the two packed heads separately in the psn matmul. A DMA'd
            # F32R version of this clogged the SDMA engines with thousands
            # of tiny descriptors (3.5us DGE) and wedged the whole head.
            onesblk = pers.tile([128, 128], BF16)
            nc.vector.memset(onesblk[:, :], 0.0)
            nc.vector.memset(onesblk[0:64, 0:64], 1.0)
            nc.vector.memset(onesblk[64:128, 64:128], 1.0)
            # causal 0/1 mask for the 128-wide diagonal strip (dup'd per u),
            # built on gpsimd (same reason: its 3-D DMA cost 7.3us of DGE)
            nc.gpsimd.memset(mask_sb[:], 1.0)
            nc.gpsimd.affine_select(
                out=mask_sb[:],
                in_=mask_sb[:],
                compare_op=mybir.AluOpType.is_ge,
                fill=0.0,
                base=0,
                channel_multiplier=-1,
                pattern=[[0, 2], [1, 128]],
            )

            # pair-stacked q (pair p holds heads 2p/2p+1 in partition halves)
            qt = [pers.tile([128, 2, QB], BF16, name=f"qt{j}") for j in range(NQB)]
            # k duplicated into both partition halves for row-group packing
            kt = [pers.tile([128, QB], BF16, name=f"kt{j}") for j in range(NQB)]
            vaug = [pers.tile([128, 4, HD + 1], BF16, name=f"va{j}") for j in range(NQB)]

            a2a_in = [
                dram.tile([N_CORES, 2, HD + 1, ROWS_PER_CORE], BF16, name=f"a2ai{p}")
                for p in range(2)
            ]
            a2a_out = [
                dram.tile([N_CORES, 2, HD + 1, ROWS_PER_CORE], BF16, name=f"a2ao{p}")
                for p in range(2)
            ]
            cc_warm_in = dram.tile([N_CORES, 4], F32, name="ccwi")
            cc_warm_out = dram.tile([N_CORES, 4], F32, name="ccwo")

            # ============ fused projections + attention ====================
            with (
                tc.tile_pool(name="xres", bufs=2) as xp,
                tc.tile_pool(name="acc", bufs=2, space="PSUM") as psA,
                tc.tile_pool(name="sc", bufs=2, space="PSUM") as psB,
                tc.tile_pool(name="pv", bufs=2, space="PSUM") as psPV,
                tc.tile_pool(name="work", bufs=2) as t1,
                tc.tile_pool(name="ptp", bufs=3) as ptp,
            ):
                xblk = {}

                def load_x(j, gate=None):
                    """SBUF-resident x for block j: [128, 16, 512] bf16, one
                    2MB load in 2 contiguous chunks (the host pre-arranges x
                    block-major so each chunk is an 8KB-per-partition run --
                    the k-tile-major layout fragmented into ~1KB descriptors
                    and clogged the SDMA engines for ~50us). `gate` (an AP
                    written by earlier compute) head-blocks the sync queue
                    via a tiny store, so these descriptors don't enter the
                    SDMA round-robin until the gate value exists."""
                    if gate is not None:
                        nc.sync.dma_start(gate_dram[:], gate)
                    xb = xp.tile([128, KT_TILES, QB], BF16, tag="x", name=f"x{j}")
                    for kk in range(0, KT_TILES, 8):
                        nc.sync.dma_start(
                            xb[:, kk : kk + 8, :], xT[:, j, kk : kk + 8, :]
                        )
                    xblk[j] = xb

                def norm_rope(j, raw_psum, idx):
                    """Evict + rmsnorm + rope one accumulator. idx 0/1 = q
                    pairs, idx 2 = kv. Generator (yields mid-chain)."""
                    sl = slice(QB * j, QB * j + QB)
                    is_kv = idx == 2
                    rows = slice(0, 64) if is_kv else slice(0, 128)
                    rawsb = t1.tile([128, QB], BF16, tag="rawsb")
                    nc.vector.tensor_copy(rawsb[:], raw_psum[:])
                    sq = t1.tile([128, QB], BF16, tag="sq")
                    nc.vector.tensor_mul(sq[:], rawsb[:], rawsb[:])
                    psn = psB.tile([128, 2, QB], F32, tag="sc", name=f"psn{idx}_{j}")
                    nc.tensor.matmul(psn[:, 0, :], onesblk[:], sq[:], start=True, stop=True)
                    lnv = t1.tile([128, QB], F32, tag="lnv", bufs=1)
                    nc.scalar.activation(
                        out=lnv[rows, :], in_=psn[rows, 0, :],
                        func=AF.Ln, bias=eps_sb[rows, :], scale=1.0 / HD,
                    )
                    rcp = t1.tile([128, QB], BF16, tag="rcp", bufs=1)
                    nc.scalar.activation(
                        out=rcp[rows, :], in_=lnv[rows, :], func=AF.Exp, scale=-0.5,
                    )
                    yield
                    tn = t1.tile([128, QB], BF16, tag="tn")
                    nc.vector.tensor_mul(tn[rows, :], rawsb[rows, :], rcp[rows, :])
                    rot = t1.tile([128, QB], BF16, tag="rot")
                    nh = 1 if is_kv else 2
                    for b in range(nh):
                        o = 64 * b
                        nc.vector.tensor_copy(rot[o : o + 32, :], tn[o + 32 : o + 64, :])
                        nc.vector.tensor_copy(rot[o + 32 : o + 64, :], tn[o : o + 32, :])
                    if is_kv:
                        tcs = t1.tile([64, QB], BF16, tag="tcs", bufs=1)
                        nc.vector.tensor_mul(tcs[:], tn[0:64, :], ck_sb[:, sl])
                        nc.vector.tensor_mul(rot[0:64, :], rot[0:64, :], sk_sb[:, sl])
                        nc.vector.tensor_add(kt[j][0:64, :], tcs[:], rot[0:64, :])
                        nc.vector.tensor_copy(kt[j][64:128, :], kt[j][0:64, :])
                        vt = t1.tile([64, QB], F32, tag="vt", bufs=1)
                        nc.vector.tensor_copy(vt[:], rawsb[64:128, :])
                        for d in range(4):
                            psv = psB.tile([128, 2, QB], F32, tag="sc", name=f"psv{j}_{d}")
                            nc.tensor.transpose(
                                psv[:, 0, 0:64],
                                vt[:, 128 * d : 128 * d + 128],
                                ident[0:64, 0:64],
                            )
                            nc.vector.tensor_copy(vaug[j][:, d, 0:HD], psv[:, 0, 0:64])
                            nc.vector.memset(vaug[j][:, d, HD : HD + 1], 1.0)
                            if d == 1:
                                yield
                    else:
                        tc2 = t1.tile([128, QB], BF16, tag="tc2")
                        nc.vector.tensor_mul(tc2[:], tn[:], cq_sb[:, sl])
                        nc.vector.tensor_mul(rot[:], rot[:], sq_sb[:, sl])
                        nc.vector.tensor_add(qt[j][:, idx, :], tc2[:], rot[:])
                    yield

                def emit_proj(j):
                    """Two-pass projection: Q (2 banks) then KV (1 bank).
                    Both passes read the SBUF-resident x block (no HBM)."""
                    xb = xblk.pop(j)
                    accq = [
                        psA.tile([128, QB], F32, tag="acc", name=f"accq{i}_{j}")
                        for i in range(2)
                    ]
                    for k in range(KT_TILES):
                        st = k == 0
                        sp = k == KT_TILES - 1
                        nc.tensor.matmul(accq[0][:], wq_sb[:, k, 0:128], xb[:, k, :], start=st, stop=sp)
                        nc.tensor.matmul(accq[1][:], wq_sb[:, k, 128:256], xb[:, k, :], start=st, stop=sp)
                        if k % 2 == 1:
                            yield
                    yield from norm_rope(j, accq[0], 0)
                    # block j+1's x load: its WAR on the xres pool slot
                    # (block j-1's matmuls) is what actually delays it --
                    # Tile schedules queues by dependency, not emission order
                    if j + 1 < NQB:
                        load_x(j + 1)
                    yield from norm_rope(j, accq[1], 1)
                    acckv = psA.tile([128, QB], F32, tag="acc", name=f"acckv_{j}")
                    for k in range(KT_TILES):
                        st = k == 0
                        sp = k == KT_TILES - 1
                        nc.tensor.matmul(acckv[:], wkv_sb[:, k, :], xb[:, k, :], start=st, stop=sp)
                        if k % 4 == 3:
                            yield
                    yield from norm_rope(j, acckv, 2)

                def emit_att_pair(j, p):
                    """Attention for block j, head pair p: scores for heads
                    2p/2p+1 run concurrently in PE row groups 0/64. Yields
                    after each unit."""
                    pvs = [
                        psPV.tile([128, QB], F32, tag="pv", name=f"pv{j}_{p}_{u}")
                        for u in range(2)
                    ]
                    ntile = 4 * j + 4
                    for t in range(ntile):
                        jj, d = t // 4, t % 4
                        diag = jj == j
                        n0 = 128 * d if diag else 0
                        w = QB - n0
                        sc = psB.tile([128, 2, QB], F32, tag="sc", name=f"sc{j}_{p}_{t}")
                        for u in range(2):
                            nc.tensor.matmul(
                                sc[:, u, 0:w],
                                kt[jj][64 * u : 64 * u + 64, 128 * d : 128 * d + 128],
                                qt[j][64 * u : 64 * u + 64, p, n0:QB],
                                start=True, stop=True,
                            )
                        pt = ptp.tile([128, 2, QB], BF16, tag="pt")
                        nc.scalar.activation(
                            out=pt[:, :, 0:w], in_=sc[:, :, 0:w],
                            func=AF.Exp, scale=0.125,
                        )
                        if diag:
                            # causal mask inside the 128-wide diagonal strip:
                            # 0/1 multiply on the bf16 exp output (2x DVE)
                            nc.vector.tensor_mul(
                                pt[:, :, 0:128], pt[:, :, 0:128], mask_sb[:]
                            )
                        for u in range(2):
                            nc.tensor.matmul(
                                pvs[u][0:65, n0:QB],
                                vaug[jj][:, d, :],
                                pt[:, u, 0:w],
                                start=(t == 0), stop=(t == ntile - 1),
                            )
                        if t % 2 == 1 or diag:
                            yield
                    for u in range(2):
                        att = t1.tile([65, QB], BF16, tag="att", bufs=8)
                        nc.vector.tensor_copy(att[:], pvs[u][0:65, :])
                        for s in range(2):
                            shard = 2 * j + s
                            cs = slice(ROWS_PER_CORE * s, ROWS_PER_CORE * (s + 1))
                            # stores ride the gpsimd queue: its FIFO order
                            # (stores -> chunk trigger -> stores -> trigger)
                            # gives the collectives exact, alias-free deps;
                            # sync-lane sem rotation was adding ~20us of
                            # false wait to the chunk-0 doorbell
                            nc.gpsimd.dma_start(a2a_in[p][shard, u, :, :], att[:, cs])
                    yield

                def chain(gens):
                    for g in gens:
                        yield from g

                def drive(gen):
                    for _ in gen:
                        pass

                def interleave(att_gen, proj_gen, att_per_proj=1):
                    att_done = proj_done = False
                    while not (att_done and proj_done):
                        for _ in range(att_per_proj):
                            if not att_done:
                                att_done = next(att_gen, "END") == "END"
                        if not proj_done:
                            proj_done = next(proj_gen, "END") == "END"

                # warmup collective, fired immediately: prepays the
                # ~25-40us first-collective ncfw setup during the early
                # compute phases (its overhead provably doesn't stall the
                # other engines), so the real chunk-0 AllToAll is cheap.
                nc.gpsimd.collective_compute(
                    "AllToAll",
                    mybir.AluOpType.bypass,
                    replica_groups=[list(range(N_CORES))],
                    ins=[cc_warm_in[:].opt()],
                    outs=[cc_warm_out[:].opt()],
                )
                load_x(0)
                load_rope_tables()
                drive(emit_proj(0))
                nc.scalar.dma_start(sel_sb[:], sel16[:])
                # pair-major attention: all blocks' pair 0 first, so the
                # pair-0 AllToAll fires as early as possible
                interleave(emit_att_pair(0, 0), emit_proj(1), att_per_proj=1)
                interleave(emit_att_pair(1, 0), emit_proj(2), att_per_proj=1)
                # wo preload: Tile hoists dependency-free DMAs to the
                # front of the queue, so a queue-position "gate" does
                # nothing. Instead create a real WAR: a tiny DVE copy of
                # qt[2] into wo_sb's first bytes forces the 8MB load to
                # wait until block-2's q rope exists, keeping it out of the
                # head DMA round-robin.
                nc.vector.tensor_copy(wo_sb[0:1, 0, 0:8], qt[2][0:1, 0, 0:8])
                nc.scalar.dma_start(wo_sb[:], wo[:])
                interleave(emit_att_pair(2, 0), emit_proj(3), att_per_proj=1)
                drive(emit_att_pair(3, 0))

                # ---- reshard chunk 0 + pair-1 attention + out-projection ----
                R = ROWS_PER_CORE
                dsb_raw = [
                    t1.tile([2 * N_CORES, R], BF16, tag=f"denraw{i}", name=f"denraw{i}", bufs=1)
                    for i in range(2)
                ]
                dsb_inv = [
                    t1.tile([2 * N_CORES, R], F32, tag=f"deninv{i}", name=f"deninv{i}", bufs=1)
                    for i in range(2)
                ]
                dsb = [
                    t1.tile([2 * N_CORES, R], F32R, tag=f"den{i}", name=f"den{i}", bufs=1)
                    for i in range(2)
                ]
                an_sb = pers.tile([128, 2 * N_CORES, R], BF16)

                def emit_chunk(p):
                    nc.gpsimd.collective_compute(
                        "AllToAll",
                        mybir.AluOpType.bypass,
                        replica_groups=[list(range(N_CORES))],
                        ins=[a2a_in[p][:].opt()],
                        outs=[a2a_out[p][:].opt()],
                    )

                def emit_den(half):
                    # on sync: the sync queue is otherwise idle in phase C
                    # (stores moved to gpsimd), so these chunk-gated loads
                    # head-block nothing
                    for u in range(2):
                        nc.sync.dma_start(
                            dsb_raw[half][8 * u : 8 * u + 8, :],
                            a2a_out[half][:, u, 64, :],
                        )

                def an_half(half, bc_pool, bc_tag):
                    """Generator: normalize the attnT slices for chunk
                    `half` (one unit per source core)."""
                    dcast = t1.tile([2 * N_CORES, R], F32, tag=f"dcast{half}",
                                    name=f"dcast{half}", bufs=1)
                    nc.vector.tensor_copy(dcast[:], dsb_raw[half][:, :])
                    nc.vector.reciprocal_approx_fast(
                        out=dsb_inv[half][:, :], in_=dcast[:]
                    )
                    nc.vector.tensor_copy(dsb[half][:, :], dsb_inv[half][:, :])
                    for g in range(N_CORES):
                        a_raw = t1.tile([128, R], BF16, tag="araw")
                        nc.sync.dma_start(
                            a_raw[0:64, :], a2a_out[half][g, 0, 0:64, :]
                        )
                        nc.sync.dma_start(
                            a_raw[64:128, :], a2a_out[half][g, 1, 0:64, :]
                        )
                        bc = bc_pool.tile(
                            [128, QB], F32, tag=bc_tag, name=f"bc{half}_{g}"
                        )
                        nc.tensor.matmul(
                            bc[:, 0:R],
                            sel_sb[:, 2 * g + half, :],
                            dsb[half][:, :],
                            start=True, stop=True,
                        )
                        nc.vector.tensor_mul(
                            an_sb[:, 2 * g + half, :], a_raw[:], bc[:, 0:R]
                        )
                        yield

                emit_chunk(0)
                emit_den(0)
                # pair-1 attention for blocks 0-2 runs while chunk 0
                # transfers; an0 (gated on chunk 0) only joins for the last
                # block so its DVE ops can't head-block the diag masks
                drive(chain([emit_att_pair(j, 1) for j in range(NQB - 1)]))
                an0 = an_half(0, psA, "acc")
                interleave(emit_att_pair(NQB - 1, 1), an0, att_per_proj=2)
                drive(an0)
                emit_chunk(1)
                emit_den(1)

                # out projection: nb 0-2 get six accumulators (acc, pv, and
                # both halves of one sc slot) so all their half-0 work runs
                # under the chunk-1 transfer; the an-half-1 bc matmuls use
                # the second sc slot; nb3 runs last in the acc slots.
                poA = [psA.tile([128, QB], F32, tag="acc", name=f"poA{q}") for q in range(2)]
                poB = [psPV.tile([128, QB], F32, tag="pv", name=f"poB{q}") for q in range(2)]
                poCt = psB.tile([128, 2, QB], F32, tag="sc", name="poC")
                po_aps = {
                    0: [poA[0][:], poA[1][:]],
                    1: [poB[0][:], poB[1][:]],
                    2: [poCt[:, 0, :], poCt[:, 1, :]],
                }

                def po_mm(nb, q, gh, first, last):
                    nc.tensor.matmul(
                        po_aps[nb][q],
                        an_sb[:, gh, 128 * q : 128 * q + 128],
                        wo_sb[:, gh, 512 * nb : 512 * nb + 512],
                        start=first, stop=last,
                    )

                for g in range(N_CORES):
                    for nb in range(3):
                        for q in range(2):
                            po_mm(nb, q, 2 * g, g == 0, False)
                an1 = an_half(1, psB, "sc")
                for g in range(N_CORES):
                    next(an1, None)
                    for nb in range(3):
                        for q in range(2):
                            po_mm(nb, q, 2 * g + 1, False, g == N_CORES - 1)
                drive(an1)

                def po_evict(nb):
                    for q in range(2):
                        osb = t1.tile([128, QB], BF16, tag="osb")
                        nc.vector.tensor_copy(osb[:], po_aps[nb][q])
                        nc.sync.dma_start(
                            out[128 * q : 128 * q + 128, 512 * nb : 512 * nb + 512],
                            osb[:],
                        )

                po_evict(0)
                po_last = [psA.tile([128, QB], F32, tag="acc", name=f"poD{q}") for q in range(2)]
                po_aps[3] = [po_last[0][:], po_last[1][:]]
                for half in range(2):
                    for g in range(N_CORES):
                        gh = 2 * g + half
                        for q in range(2):
                            po_mm(3, q, gh, half == 0 and g == 0,
                                  half == 1 and g == N_CORES - 1)
                po_evict(1)
                po_evict(2)
                po_evict(3)

    nc.compile()
    return nc


_NC_CACHE = None


def _get_nc():
    global _NC_CACHE
    if _NC_CACHE is None:
        _NC_CACHE = _build()
    return _NC_CACHE


def _to_ktile_layout(w):
    m = w.shape[1]
    return np.ascontiguousarray(w.reshape(KT_TILES, 128, m).transpose(1, 0, 2))


def _make_in_maps(x, cos, sin, wq, wk, wv, wo, q_norm_w, k_norm_w):
    x = np.asarray(x, dtype=np.float32)
    cos = np.asarray(cos, dtype=np.float32)
    sin = np.asarray(sin, dtype=np.float32)
    wq = np.asarray(wq, dtype=np.float32)
    wk = np.asarray(wk, dtype=np.float32)
    wv = np.asarray(wv, dtype=np.float32)
    wo = np.asarray(wo, dtype=np.float32)
    qw = np.asarray(q_norm_w, dtype=np.float32)
    kw = np.asarray(k_norm_w, dtype=np.float32)

    xk = _to_ktile_layout(np.ascontiguousarray(x[0].T))  # [128, 16, 2048]
    xT = np.ascontiguousarray(
        xk.reshape(128, KT_TILES, NQB, QB).transpose(0, 2, 1, 3)
    ).astype(BF16_NP)  # [128, 4, 16, 512] block-major
    wo_b = _to_ktile_layout(wo).astype(BF16_NP)

    cosT = cos.T  # [64, SEQ]
    sinT = sin.T
    sgn = np.where(np.arange(64) < 32, -1.0, 1.0).astype(np.float32)
    wrot_q = qw[(np.arange(64) + 32) % 64]
    wrot_k = kw[(np.arange(64) + 32) % 64]
    cq1 = cosT * qw[:, None]
    sq1 = sinT * (sgn * wrot_q)[:, None]
    coswq = np.ascontiguousarray(np.vstack([cq1, cq1])).astype(BF16_NP)
    sinwq = np.ascontiguousarray(np.vstack([sq1, sq1])).astype(BF16_NP)
    coswk = np.ascontiguousarray(cosT * kw[:, None]).astype(BF16_NP)
    sinwk = np.ascontiguousarray(sinT * (sgn * wrot_k)[:, None]).astype(BF16_NP)

    sel16 = np.zeros((2 * N_CORES, 2 * N_CORES, 128), np.float32)
    for g in range(N_CORES):
        for half in range(2):
            for m in range(128):
                sel16[8 * (m // 64) + g, 2 * g + half, m] = 1.0

    in_maps = []
    for c in range(N_CORES):
        wq_c = _to_ktile_layout(
            np.ascontiguousarray(wq[:, 256 * c : 256 * c + 256])
        ).astype(BF16_NP)
        wkv_c = _to_ktile_layout(
            np.ascontiguousarray(
                np.concatenate(
                    [wk[:, 64 * c : 64 * c + 64], wv[:, 64 * c : 64 * c + 64]],
                    axis=1,
                )
            )
        ).astype(BF16_NP)
        in_maps.append(
            {
                "xT": xT,
                "wq": wq_c,
                "wkv": wkv_c,
                "wo": wo_b,
                "coswq": coswq,
                "sinwq": sinwq,
                "coswk": coswk,
                "sinwk": sinwk,
                "sel16": sel16,
            }
        )
    return in_maps


def kernel(x, cos, sin, wq, wk, wv, wo, q_norm_w, k_norm_w):
    in_maps = _make_in_maps(x, cos, sin, wq, wk, wv, wo, q_norm_w, k_norm_w)
    nc = _get_nc()
    res = run_bass_kernel_spmd(nc, in_maps, core_ids=list(range(N_CORES)))
    rows = [res.results[c]["out"] for c in range(N_CORES)]
    full = np.concatenate(rows, axis=0)  # [SEQ, D_IN]
    return full.reshape(1, SEQ, D_IN).astype(np.float32)


# revision 23
# speedup vs baseline: 1.0744x; 1.0744x over previous
"""GQA FlashAttention (RMSNorm QK + RoPE, causal) on 8 TRN2 NeuronCores.

Sharding: tensor-parallel over heads (core c owns q-heads 4c..4c+3 and
kv-head c; the GQA group is fully local). Head-pair-chunked bf16
AllToAlls re-shard the attention output from head-parallel to
row-parallel; each core then computes its 256 output rows against the
full (SBUF-resident) Wo.

v3 structure:
- x is SBUF-resident per 512-column block ([128,16,512] bf16, 2-deep):
  each block is loaded ONCE (8MB HBM traffic instead of 16MB) and both
  projection passes read SBUF, so the PE k-loops carry no DMA deps.
- x block j+1's load descriptors are GATED behind block j's q-norm via
  a tiny qt->DRAM store that head-blocks the sync queue: the 16 SDMA
  engines round-robin over every queued DMA at packet granularity, so
  without the gate the block-0-critical loads finish only when ALL
  queued head traffic finishes (~45us measured).
- Attention runs PAIR-MAJOR: all four blocks' head-pair 0 first, so
  the pair-0 AllToAll (532KB, ~20-30us) fires ~45us earlier and
  transfers entirely under pair-1's attention; the pair-1 AllToAll is
  covered by the even-half of the out-projection.
- Diagonal causal mask: 0/1 bf16 multiply on the exp output (SBUF,
  2x DVE mode) instead of -1e9 f32 adds on PSUM scores.
- Norm chain evictions in bf16 (squares still computed from f32 PSUM);
  output stored bf16 and upcast on the host.
- bf16 operands everywhere on the PE, fp32 accumulation in PSUM;
  fused emission keeps the in-order PE stream dense (HAM stays warm);
  row-packed score matmuls (heads 2p/2p+1 in PE row groups 0/64);
  batched softmax ([128,2,512] exp ACTIVATEs, denominators ride a
  ones-column in V, normalization folded to reciprocal+broadcast on
  the re-sharded output); rsqrt = exp(-0.5*ln(var+eps)) so the whole
  kernel needs ONE activation table load.
"""

import sys

sys.path.insert(0, "/opt/trn_rl_repo")

import numpy as np
import ml_dtypes
import concourse.bass as bass  # noqa: F401
import concourse.tile as tile
from concourse import mybir, bacc
import concourse.bacc as bacc_mod
from concourse.bass_utils import run_bass_kernel_spmd
from concourse.hw_specs import get_activation_tables as _orig_get_tables
from concourse.masks import make_identity

N_CORES = 8
D_IN = 2048
SEQ = 2048
N_HEADS = 32
N_KV = 8
HD = 64
HPC = N_HEADS // N_CORES  # 4 q heads per core
EPS = 1e-6

F32 = mybir.dt.float32
F32R = mybir.dt.float32r
BF16 = mybir.dt.bfloat16
BF16_NP = ml_dtypes.bfloat16

KT_TILES = D_IN // 128
QB = 512
NQB = SEQ // QB  # 4
ROWS_PER_CORE = SEQ // N_CORES  # 256
AF = mybir.ActivationFunctionType

_ONE_TABLE = "natural_log_exp_and_others"


def _pinned_tables(arch):
    tabs = _orig_get_tables(arch)
    return {n: (fs if n == _ONE_TABLE else set()) for n, fs in tabs.items()}


def _build():
    bacc_mod.get_activation_tables = _pinned_tables
    nc = bacc.Bacc(num_devices=N_CORES)

    xT = nc.dram_tensor("xT", [128, NQB, KT_TILES, QB], BF16, kind="ExternalInput")
    wq = nc.dram_tensor("wq", [128, KT_TILES, HPC * HD], BF16, kind="ExternalInput")
    wkv = nc.dram_tensor("wkv", [128, KT_TILES, 2 * HD], BF16, kind="ExternalInput")
    wo = nc.dram_tensor("wo", [128, KT_TILES, D_IN], BF16, kind="ExternalInput")
    coswq = nc.dram_tensor("coswq", [128, SEQ], BF16, kind="ExternalInput")
    sinwq = nc.dram_tensor("sinwq", [128, SEQ], BF16, kind="ExternalInput")
    coswk = nc.dram_tensor("coswk", [64, SEQ], BF16, kind="ExternalInput")
    sinwk = nc.dram_tensor("sinwk", [64, SEQ], BF16, kind="ExternalInput")
    sel16 = nc.dram_tensor("sel16", [2 * N_CORES, 2 * N_CORES, 128], F32R, kind="ExternalInput")

    out = nc.dram_tensor("out", [ROWS_PER_CORE, D_IN], BF16, kind="ExternalOutput")

    with tile.TileContext(nc) as tc:
        with (
            tc.tile_pool(name="persist", bufs=1) as pers,
            tc.tile_pool(name="dram", bufs=1, space="DRAM") as dram,
        ):
            # ---- persistent SBUF preloads (contiguous, host-transposed) ----
            # weights preload from the (otherwise idle) scalar queue so the
            # sync queue starts issuing x chunks immediately
            wq_sb = pers.tile([128, KT_TILES, HPC * HD], BF16)
            wkv_sb = pers.tile([128, KT_TILES, 2 * HD], BF16)
            nc.scalar.dma_start(wq_sb[:, 0:8, :], wq[:, 0:8, :])
            nc.scalar.dma_start(wkv_sb[:], wkv[:])
            nc.scalar.dma_start(wq_sb[:, 8:16, :], wq[:, 8:16, :])

            cq_sb = pers.tile([128, SEQ], BF16)
            sq_sb = pers.tile([128, SEQ], BF16)
            ck_sb = pers.tile([64, SEQ], BF16)
            sk_sb = pers.tile([64, SEQ], BF16)
            mask_sb = pers.tile([128, 2, 128], BF16)

            def load_rope_tables():
                # issued on the sync queue AFTER block-0's x chunks: in-order
                # issue keeps the first matmuls' data ahead of these
                nc.sync.dma_start(ck_sb[:], coswk[:])
                nc.sync.dma_start(sk_sb[:], sinwk[:])
                nc.sync.dma_start(cq_sb[:], coswq[:])
                nc.sync.dma_start(sq_sb[:], sinwq[:])

            wo_sb = pers.tile([128, KT_TILES, D_IN], BF16)  # 8 MB
            sel_sb = pers.tile([2 * N_CORES, 2 * N_CORES, 128], F32R)

            ident = pers.tile([128, 128], F32)
            make_identity(nc, ident[:])
            eps_sb = pers.tile([128, 1], F32)
            nc.vector.memset(eps_sb[:], EPS)
            # block-diagonal ones (64x64 blocks), bf16, built on-chip: sums
            # the two packed heads separately in the psn matmul. A DMA'd
            # F32R version of this clogged the SDMA engines with thousands
            # of tiny descriptors (3.5us DGE) and wedged the whole head.
            onesblk = pers.tile([128, 128], BF16)
            nc.vector.memset(onesblk[:, :], 0.0)
            nc.vector.memset(onesblk[0:64, 0:64], 1.0)
            nc.vector.memset(onesblk[64:128, 64:128], 1.0)
            # causal 0/1 mask for the 128-wide diagonal strip (dup'd per u),
            # built on gpsimd (same reason: its 3-D DMA cost 7.3us of DGE)
            nc.gpsimd.memset(mask_sb[:], 1.0)
            nc.gpsimd.affine_select(
                out=mask_sb[:],
                in_=mask_sb[:],
                compare_op=mybir.AluOpType.is_ge,
                fill=0.0,
                base=0,
                channel_multiplier=-1,
                pattern=[[0, 2], [1, 128]],
            )

            # pair-stacked q (pair p holds heads 2p/2p+1 in partition halves)
            qt = [pers.tile([128, 2, QB], BF16, name=f"qt{j}") for j in range(NQB)]
            # k duplicated into both partition halves for row-group packing
            kt = [pers.tile([128, QB], BF16, name=f"kt{j}") for j in range(NQB)]
            vaug = [pers.tile([128, 4, HD + 1], BF16, name=f"va{j}") for j in range(NQB)]

            a2a_in = [
                dram.tile([N_CORES, 2, HD + 1, ROWS_PER_CORE], BF16, name=f"a2ai{p}")
                for p in range(2)
            ]
            a2a_out = [
                dram.tile([N_CORES, 2, HD + 1, ROWS_PER_CORE], BF16, name=f"a2ao{p}")
                for p in range(2)
            ]
            cc_warm_in = dram.tile([N_CORES, 4], F32, name="ccwi")
            cc_warm_out = dram.tile([N_CORES, 4], F32, name="ccwo")

            # ============ fused projections + attention ====================
            with (
                tc.tile_pool(name="xres", bufs=2) as xp,
                tc.tile_pool(name="acc", bufs=2, space="PSUM") as psA,
                tc.tile_pool(name="sc", bufs=2, space="PSUM") as psB,
                tc.tile_pool(name="pv", bufs=2, space="PSUM") as psPV,
                tc.tile_pool(name="work", bufs=2) as t1,
                tc.tile_pool(name="ptp", bufs=3) as ptp,
            ):
                xblk = {}

                def load_x(j, gate=None):
                    """SBUF-resident x for block j: [128, 16, 512] bf16, one
                    2MB load in 2 contiguous chunks (the host pre-arranges x
                    block-major so each chunk is an 8KB-per-partition run --
                    the k-tile-major layout fragmented into ~1KB descriptors
                    and clogged the SDMA engines for ~50us). `gate` (an AP
                    written by earlier compute) head-blocks the sync queue
                    via a tiny store, so these descriptors don't enter the
                    SDMA round-robin until the gate value exists."""
                    if gate is not None:
                        nc.sync.dma_start(gate_dram[:], gate)
                    xb = xp.tile([128, KT_TILES, QB], BF16, tag="x", name=f"x{j}")
                    for kk in range(0, KT_TILES, 8):
                        nc.sync.dma_start(
                            xb[:, kk : kk + 8, :], xT[:, j, kk : kk + 8, :]
                        )
                    xblk[j] = xb

                def norm_rope(j, raw_psum, idx):
                    """Evict + rmsnorm + rope one accumulator. idx 0/1 = q
                    pairs, idx 2 = kv. Generator (yields mid-chain)."""
                    sl = slice(QB * j, QB * j + QB)
                    is_kv = idx == 2
                    rows = slice(0, 64) if is_kv else slice(0, 128)
                    rawsb = t1.tile([128, QB], BF16, tag="rawsb")
                    nc.vector.tensor_copy(rawsb[:], raw_psum[:])
                    sq = t1.tile([128, QB], BF16, tag="sq")
                    nc.vector.tensor_mul(sq[:], rawsb[:], rawsb[:])
                    psn = psB.tile([128, 2, QB], F32, tag="sc", name=f"psn{idx}_{j}")
                    nc.tensor.matmul(psn[:, 0, :], onesblk[:], sq[:], start=True, stop=True)
                    lnv = t1.tile([128, QB], F32, tag="lnv", bufs=1)
                    nc.scalar.activation(
                        out=lnv[rows, :], in_=psn[rows, 0, :],
                        func=AF.Ln, bias=eps_sb[rows, :], scale=1.0 / HD,
                    )
                    rcp = t1.tile([128, QB], BF16, tag="rcp", bufs=1)
                    nc.scalar.activation(
                        out=rcp[rows, :], in_=lnv[rows, :], func=AF.Exp, scale=-0.5,
                    )
                    yield
                    tn = t1.tile([128, QB], BF16, tag="tn")
                    nc.vector.tensor_mul(tn[rows, :], rawsb[rows, :], rcp[rows, :])
                    rot = t1.tile([128, QB], BF16, tag="rot")
                    nh = 1 if is_kv else 2
                    for b in range(nh):
                        o = 64 * b
                        nc.vector.tensor_copy(rot[o : o + 32, :], tn[o + 32 : o + 64, :])
                        nc.vector.tensor_copy(rot[o + 32 : o + 64, :], tn[o : o + 32, :])
                    if is_kv:
                        tcs = t1.tile([64, QB], BF16, tag="tcs", bufs=1)
                        nc.vector.tensor_mul(tcs[:], tn[0:64, :], ck_sb[:, sl])
                        nc.vector.tensor_mul(rot[0:64, :], rot[0:64, :], sk_sb[:, sl])
                        nc.vector.tensor_add(kt[j][0:64, :], tcs[:], rot[0:64, :])
                        nc.vector.tensor_copy(kt[j][64:128, :], kt[j][0:64, :])
                        vt = t1.tile([64, QB], F32, tag="vt", bufs=1)
                        nc.vector.tensor_copy(vt[:], rawsb[64:128, :])
                        for d in range(4):
                            psv = psB.tile([128, 2, QB], F32, tag="sc", name=f"psv{j}_{d}")
                            nc.tensor.transpose(
                                psv[:, 0, 0:64],
                                vt[:, 128 * d : 128 * d + 128],
                                ident[0:64, 0:64],
                            )
                            nc.vector.tensor_copy(vaug[j][:, d, 0:HD], psv[:, 0, 0:64])
                            nc.vector.memset(vaug[j][:, d, HD : HD + 1], 1.0)
                            if d == 1:
                                yield
                    else:
                        tc2 = t1.tile([128, QB], BF16, tag="tc2")
                        nc.vector.tensor_mul(tc2[:], tn[:], cq_sb[:, sl])
                        nc.vector.tensor_mul(rot[:], rot[:], sq_sb[:, sl])
                        nc.vector.tensor_add(qt[j][:, idx, :], tc2[:], rot[:])
                    yield

                def emit_proj(j):
                    """Two-pass projection: Q (2 banks) then KV (1 bank).
                    Both passes read the SBUF-resident x block (no HBM)."""
                    xb = xblk.pop(j)
                    accq = [
                        psA.tile([128, QB], F32, tag="acc", name=f"accq{i}_{j}")
                        for i in range(2)
                    ]
                    for k in range(KT_TILES):
                        st = k == 0
                        sp = k == KT_TILES - 1
                        nc.tensor.matmul(accq[0][:], wq_sb[:, k, 0:128], xb[:, k, :], start=st, stop=sp)
                        nc.tensor.matmul(accq[1][:], wq_sb[:, k, 128:256], xb[:, k, :], start=st, stop=sp)
                        if k % 2 == 1:
                            yield
                    yield from norm_rope(j, accq[0], 0)
                    # block j+1's x load: its WAR on the xres pool slot
                    # (block j-1's matmuls) is what actually delays it --
                    # Tile schedules queues by dependency, not emission order
                    if j + 1 < NQB:
                        load_x(j + 1)
                    yield from norm_rope(j, accq[1], 1)
                    acckv = psA.tile([128, QB], F32, tag="acc", name=f"acckv_{j}")
                    for k in range(KT_TILES):
                        st = k == 0
                        sp = k == KT_TILES - 1
                        nc.tensor.matmul(acckv[:], wkv_sb[:, k, :], xb[:, k, :], start=st, stop=sp)
                        if k % 4 == 3:
                            yield
                    yield from norm_rope(j, acckv, 2)

                def emit_att_pair(j, p):
                    """Attention for block j, head pair p: scores for heads
                    2p/2p+1 run concurrently in PE row groups 0/64. Yields
                    after each unit."""
                    pvs = [
                        psPV.tile([128, QB], F32, tag="pv", name=f"pv{j}_{p}_{u}")
                        for u in range(2)
                    ]
                    ntile = 4 * j + 4
                    for t in range(ntile):
                        jj, d = t // 4, t % 4
                        diag = jj == j
                        n0 = 128 * d if diag else 0
                        w = QB - n0
                        sc = psB.tile([128, 2, QB], F32, tag="sc", name=f"sc{j}_{p}_{t}")
                        for u in range(2):
                            nc.tensor.matmul(
                                sc[:, u, 0:w],
                                kt[jj][64 * u : 64 * u + 64, 128 * d : 128 * d + 128],
                                qt[j][64 * u : 64 * u + 64, p, n0:QB],
                                start=True, stop=True,
                            )
                        pt = ptp.tile([128, 2, QB], BF16, tag="pt")
                        nc.scalar.activation(
                            out=pt[:, :, 0:w], in_=sc[:, :, 0:w],
                            func=AF.Exp, scale=0.125,
                        )
                        if diag:
                            # causal mask inside the 128-wide diagonal strip:
                            # 0/1 multiply on the bf16 exp output (2x DVE)
                            nc.vector.tensor_mul(
                                pt[:, :, 0:128], pt[:, :, 0:128], mask_sb[:]
                            )
                        for u in range(2):
                            nc.tensor.matmul(
                                pvs[u][0:65, n0:QB],
                                vaug[jj][:, d, :],
                                pt[:, u, 0:w],
                                start=(t == 0), stop=(t == ntile - 1),
                            )
                        if t % 2 == 1 or diag:
                            yield
                    for u in range(2):
                        att = t1.tile([65, QB], BF16, tag="att", bufs=8)
                        nc.vector.tensor_copy(att[:], pvs[u][0:65, :])
                        # late stores ride the gpsimd queue: its FIFO
                        # order (stores -> chunk trigger -> stores ->
                        # trigger) gives the collectives exact alias-free
                        # deps (sync-lane sem rotation added ~20us of false
                        # wait to the chunk-0 doorbell). Early pair-0 stores
                        # stay on sync: the gpsimd queue is blocked by the
                        # warmup collective's completion wait until ~90us.
                        eng = nc.gpsimd if (p == 1 or j == 3) else nc.sync
                        for s in range(2):
                            shard = 2 * j + s
                            cs = slice(ROWS_PER_CORE * s, ROWS_PER_CORE * (s + 1))
                            eng.dma_start(a2a_in[p][shard, u, :, :], att[:, cs])
                    yield

                def chain(gens):
                    for g in gens:
                        yield from g

                def drive(gen):
                    for _ in gen:
                        pass

                def interleave(att_gen, proj_gen, att_per_proj=1):
                    att_done = proj_done = False
                    while not (att_done and proj_done):
                        for _ in range(att_per_proj):
                            if not att_done:
                                att_done = next(att_gen, "END") == "END"
                        if not proj_done:
                            proj_done = next(proj_gen, "END") == "END"

                # warmup collective, fired immediately: prepays the
                # ~25-40us first-collective ncfw setup during the early
                # compute phases (its overhead provably doesn't stall the
                # other engines), so the real chunk-0 AllToAll is cheap.
                nc.gpsimd.collective_compute(
                    "AllToAll",
                    mybir.AluOpType.bypass,
                    replica_groups=[list(range(N_CORES))],
                    ins=[cc_warm_in[:].opt()],
                    outs=[cc_warm_out[:].opt()],
                )
                load_x(0)
                load_rope_tables()
                drive(emit_proj(0))
                nc.scalar.dma_start(sel_sb[:], sel16[:])
                # pair-major attention: all blocks' pair 0 first, so the
                # pair-0 AllToAll fires as early as possible
                interleave(emit_att_pair(0, 0), emit_proj(1), att_per_proj=1)
                interleave(emit_att_pair(1, 0), emit_proj(2), att_per_proj=1)
                # wo preload: Tile hoists dependency-free DMAs to the
                # front of the queue, so a queue-position "gate" does
                # nothing. Instead create a real WAR: a tiny DVE copy of
                # qt[2] into wo_sb's first bytes forces the 8MB load to
                # wait until block-2's q rope exists, keeping it out of the
                # head DMA round-robin.
                nc.vector.tensor_copy(wo_sb[0:1, 0, 0:8], qt[2][0:1, 0, 0:8])
                nc.scalar.dma_start(wo_sb[:], wo[:])
                interleave(emit_att_pair(2, 0), emit_proj(3), att_per_proj=1)
                drive(emit_att_pair(3, 0))

                # ---- reshard chunk 0 + pair-1 attention + out-projection ----
                R = ROWS_PER_CORE
                dsb_raw = [
                    t1.tile([2 * N_CORES, R], BF16, tag=f"denraw{i}", name=f"denraw{i}", bufs=1)
                    for i in range(2)
                ]
                dsb_inv = [
                    t1.tile([2 * N_CORES, R], F32, tag=f"deninv{i}", name=f"deninv{i}", bufs=1)
                    for i in range(2)
                ]
                dsb = [
                    t1.tile([2 * N_CORES, R], F32R, tag=f"den{i}", name=f"den{i}", bufs=1)
                    for i in range(2)
                ]
                an_sb = pers.tile([128, 2 * N_CORES, R], BF16)

                def emit_chunk(p):
                    nc.gpsimd.collective_compute(
                        "AllToAll",
                        mybir.AluOpType.bypass,
                        replica_groups=[list(range(N_CORES))],
                        ins=[a2a_in[p][:].opt()],
                        outs=[a2a_out[p][:].opt()],
                    )

                def emit_den(half):
                    # on sync: the sync queue is otherwise idle in phase C
                    # (stores moved to gpsimd), so these chunk-gated loads
                    # head-block nothing
                    for u in range(2):
                        nc.sync.dma_start(
                            dsb_raw[half][8 * u : 8 * u + 8, :],
                            a2a_out[half][:, u, 64, :],
                        )

                def an_half(half, bc_pool, bc_tag):
                    """Generator: normalize the attnT slices for chunk
                    `half` (one unit per source core)."""
                    dcast = t1.tile([2 * N_CORES, R], F32, tag=f"dcast{half}",
                                    name=f"dcast{half}", bufs=1)
                    nc.vector.tensor_copy(dcast[:], dsb_raw[half][:, :])
                    nc.vector.reciprocal_approx_fast(
                        out=dsb_inv[half][:, :], in_=dcast[:]
                    )
                    nc.vector.tensor_copy(dsb[half][:, :], dsb_inv[half][:, :])
                    for g in range(N_CORES):
                        a_raw = t1.tile([128, R], BF16, tag="araw")
                        nc.sync.dma_start(
                            a_raw[0:64, :], a2a_out[half][g, 0, 0:64, :]
                        )
                        nc.sync.dma_start(
                            a_raw[64:128, :], a2a_out[half][g, 1, 0:64, :]
                        )
                        bc = bc_pool.tile(
                            [128, QB], F32, tag=bc_tag, name=f"bc{half}_{g}"
                        )
                        nc.tensor.matmul(
                            bc[:, 0:R],
                            sel_sb[:, 2 * g + half, :],
                            dsb[half][:, :],
                            start=True, stop=True,
                        )
                        nc.vector.tensor_mul(
                            an_sb[:, 2 * g + half, :], a_raw[:], bc[:, 0:R]
                        )
                        yield

                emit_chunk(0)
                emit_den(0)
                # pair-1 attention for blocks 0-2 runs while chunk 0
                # transfers; an0 (gated on chunk 0) only joins for the last
                # block so its DVE ops can't head-block the diag masks
                drive(chain([emit_att_pair(j, 1) for j in range(NQB - 1)]))
                an0 = an_half(0, psA, "acc")
                interleave(emit_att_pair(NQB - 1, 1), an0, att_per_proj=2)
                drive(an0)
                emit_chunk(1)
                emit_den(1)

                # out projection: nb 0-2 get six accumulators (acc, pv, and
                # both halves of one sc slot) so all their half-0 work runs
                # under the chunk-1 transfer; the an-half-1 bc matmuls use
                # the second sc slot; nb3 runs last in the acc slots.
                poA = [psA.tile([128, QB], F32, tag="acc", name=f"poA{q}") for q in range(2)]
                poB = [psPV.tile([128, QB], F32, tag="pv", name=f"poB{q}") for q in range(2)]
                poCt = psB.tile([128, 2, QB], F32, tag="sc", name="poC")
                po_aps = {
                    0: [poA[0][:], poA[1][:]],
                    1: [poB[0][:], poB[1][:]],
                    2: [poCt[:, 0, :], poCt[:, 1, :]],
                }

                def po_mm(nb, q, gh, first, last):
                    nc.tensor.matmul(
                        po_aps[nb][q],
                        an_sb[:, gh, 128 * q : 128 * q + 128],
                        wo_sb[:, gh, 512 * nb : 512 * nb + 512],
                        start=first, stop=last,
                    )

                for g in range(N_CORES):
                    for nb in range(3):
                        for q in range(2):
                            po_mm(nb, q, 2 * g, g == 0, False)
                an1 = an_half(1, psB, "sc")
                for g in range(N_CORES):
                    next(an1, None)
                    for nb in range(3):
                        for q in range(2):
                            po_mm(nb, q, 2 * g + 1, False, g == N_CORES - 1)
                drive(an1)

                def po_evict(nb):
                    for q in range(2):
                        osb = t1.tile([128, QB], BF16, tag="osb")
                        nc.vector.tensor_copy(osb[:], po_aps[nb][q])
                        nc.sync.dma_start(
                            out[128 * q : 128 * q + 128, 512 * nb : 512 * nb + 512],
                            osb[:],
                        )

                po_evict(0)
                po_last = [psA.tile([128, QB], F32, tag="acc", name=f"poD{q}") for q in range(2)]
                po_aps[3] = [po_last[0][:], po_last[1][:]]
                for half in range(2):
                    for g in range(N_CORES):
                        gh = 2 * g + half
                        for q in range(2):
                            po_mm(3, q, gh, half == 0 and g == 0,
                                  half == 1 and g == N_CORES - 1)
                po_evict(1)
                po_evict(2)
                po_evict(3)

    nc.compile()
    return nc


_NC_CACHE = None


def _get_nc():
    global _NC_CACHE
    if _NC_CACHE is None:
        _NC_CACHE = _build()
    return _NC_CACHE


def _to_ktile_layout(w):
    m = w.shape[1]
    return np.ascontiguousarray(w.reshape(KT_TILES, 128, m).transpose(1, 0, 2))


def _make_in_maps(x, cos, sin, wq, wk, wv, wo, q_norm_w, k_norm_w):
    x = np.asarray(x, dtype=np.float32)
    cos = np.asarray(cos, dtype=np.float32)
    sin = np.asarray(sin, dtype=np.float32)
    wq = np.asarray(wq, dtype=np.float32)
    wk = np.asarray(wk, dtype=np.float32)
    wv = np.asarray(wv, dtype=np.float32)
    wo = np.asarray(wo, dtype=np.float32)
    qw = np.asarray(q_norm_w, dtype=np.float32)
    kw = np.asarray(k_norm_w, dtype=np.float32)

    xk = _to_ktile_layout(np.ascontiguousarray(x[0].T))  # [128, 16, 2048]
    xT = np.ascontiguousarray(
        xk.reshape(128, KT_TILES, NQB, QB).transpose(0, 2, 1, 3)
    ).astype(BF16_NP)  # [128, 4, 16, 512] block-major
    wo_b = _to_ktile_layout(wo).astype(BF16_NP)

    cosT = cos.T  # [64, SEQ]
    sinT = sin.T
    sgn = np.where(np.arange(64) < 32, -1.0, 1.0).astype(np.float32)
    wrot_q = qw[(np.arange(64) + 32) % 64]
    wrot_k = kw[(np.arange(64) + 32) % 64]
    cq1 = cosT * qw[:, None]
    sq1 = sinT * (sgn * wrot_q)[:, None]
    coswq = np.ascontiguousarray(np.vstack([cq1, cq1])).astype(BF16_NP)
    sinwq = np.ascontiguousarray(np.vstack([sq1, sq1])).astype(BF16_NP)
    coswk = np.ascontiguousarray(cosT * kw[:, None]).astype(BF16_NP)
    sinwk = np.ascontiguousarray(sinT * (sgn * wrot_k)[:, None]).astype(BF16_NP)

    sel16 = np.zeros((2 * N_CORES, 2 * N_CORES, 128), np.float32)
    for g in range(N_CORES):
        for half in range(2):
            for m in range(128):
                sel16[8 * (m // 64) + g, 2 * g + half, m] = 1.0

    in_maps = []
    for c in range(N_CORES):
        wq_c = _to_ktile_layout(
            np.ascontiguousarray(wq[:, 256 * c : 256 * c + 256])
        ).astype(BF16_NP)
        wkv_c = _to_ktile_layout(
            np.ascontiguousarray(
                np.concatenate(
                    [wk[:, 64 * c : 64 * c + 64], wv[:, 64 * c : 64 * c + 64]],
                    axis=1,
                )
            )
        ).astype(BF16_NP)
        in_maps.append(
            {
                "xT": xT,
                "wq": wq_c,
                "wkv": wkv_c,
                "wo": wo_b,
                "coswq": coswq,
                "sinwq": sinwq,
                "coswk": coswk,
                "sinwk": sinwk,
                "sel16": sel16,
            }
        )
    return in_maps


def kernel(x, cos, sin, wq, wk, wv, wo, q_norm_w, k_norm_w):
    in_maps = _make_in_maps(x, cos, sin, wq, wk, wv, wo, q_norm_w, k_norm_w)
    nc = _get_nc()
    res = run_bass_kernel_spmd(nc, in_maps, core_ids=list(range(N_CORES)))
    rows = [res.results[c]["out"] for c in range(N_CORES)]
    full = np.concatenate(rows, axis=0)  # [SEQ, D_IN]
    return full.reshape(1, SEQ, D_IN).astype(np.float32)


# revision 24
# speedup vs baseline: 1.1380x; 1.0593x over previous
"""GQA FlashAttention (RMSNorm QK + RoPE, causal) on 8 TRN2 NeuronCores.

Sharding: tensor-parallel over heads (core c owns q-heads 4c..4c+3 and
kv-head c; the GQA group is fully local). Head-pair-chunked bf16
AllToAlls re-shard the attention output from head-parallel to
row-parallel; each core then computes its 256 output rows against the
full (SBUF-resident) Wo.

v3 structure:
- x is SBUF-resident per 512-column block ([128,16,512] bf16, 2-deep):
  each block is loaded ONCE (8MB HBM traffic instead of 16MB) and both
  projection passes read SBUF, so the PE k-loops carry no DMA deps.
- x block j+1's load descriptors are GATED behind block j's q-norm via
  a tiny qt->DRAM store that head-blocks the sync queue: the 16 SDMA
  engines round-robin over every queued DMA at packet granularity, so
  without the gate the block-0-critical loads finish only when ALL
  queued head traffic finishes (~45us measured).
- Attention runs PAIR-MAJOR: all four blocks' head-pair 0 first, so
  the pair-0 AllToAll (532KB, ~20-30us) fires ~45us earlier and
  transfers entirely under pair-1's attention; the pair-1 AllToAll is
  covered by the even-half of the out-projection.
- Diagonal causal mask: 0/1 bf16 multiply on the exp output (SBUF,
  2x DVE mode) instead of -1e9 f32 adds on PSUM scores.
- Norm chain evictions in bf16 (squares still computed from f32 PSUM);
  output stored bf16 and upcast on the host.
- bf16 operands everywhere on the PE, fp32 accumulation in PSUM;
  fused emission keeps the in-order PE stream dense (HAM stays warm);
  row-packed score matmuls (heads 2p/2p+1 in PE row groups 0/64);
  batched softmax ([128,2,512] exp ACTIVATEs, denominators ride a
  ones-column in V, normalization folded to reciprocal+broadcast on
  the re-sharded output); rsqrt = exp(-0.5*ln(var+eps)) so the whole
  kernel needs ONE activation table load.
"""

import sys

sys.path.insert(0, "/opt/trn_rl_repo")

import numpy as np
import ml_dtypes
import concourse.bass as bass  # noqa: F401
import concourse.tile as tile
from concourse import mybir, bacc
import concourse.bacc as bacc_mod
from concourse.bass_utils import run_bass_kernel_spmd
from concourse.hw_specs import get_activation_tables as _orig_get_tables
from concourse.masks import make_identity

N_CORES = 8
D_IN = 2048
SEQ = 2048
N_HEADS = 32
N_KV = 8
HD = 64
HPC = N_HEADS // N_CORES  # 4 q heads per core
EPS = 1e-6

F32 = mybir.dt.float32
F32R = mybir.dt.float32r
BF16 = mybir.dt.bfloat16
BF16_NP = ml_dtypes.bfloat16

KT_TILES = D_IN // 128
QB = 512
NQB = SEQ // QB  # 4
ROWS_PER_CORE = SEQ // N_CORES  # 256
AF = mybir.ActivationFunctionType

_ONE_TABLE = "natural_log_exp_and_others"


def _pinned_tables(arch):
    tabs = _orig_get_tables(arch)
    return {n: (fs if n == _ONE_TABLE else set()) for n, fs in tabs.items()}


def _build():
    bacc_mod.get_activation_tables = _pinned_tables
    nc = bacc.Bacc(num_devices=N_CORES)

    xT = nc.dram_tensor("xT", [128, NQB, KT_TILES, QB], BF16, kind="ExternalInput")
    wq = nc.dram_tensor("wq", [128, KT_TILES, HPC * HD], BF16, kind="ExternalInput")
    wkv = nc.dram_tensor("wkv", [128, KT_TILES, 2 * HD], BF16, kind="ExternalInput")
    wo = nc.dram_tensor("wo", [128, KT_TILES, D_IN], BF16, kind="ExternalInput")
    coswq = nc.dram_tensor("coswq", [128, SEQ], BF16, kind="ExternalInput")
    sinwq = nc.dram_tensor("sinwq", [128, SEQ], BF16, kind="ExternalInput")
    coswk = nc.dram_tensor("coswk", [64, SEQ], BF16, kind="ExternalInput")
    sinwk = nc.dram_tensor("sinwk", [64, SEQ], BF16, kind="ExternalInput")
    sel16 = nc.dram_tensor("sel16", [2 * N_CORES, 2 * N_CORES, 128], F32R, kind="ExternalInput")

    out = nc.dram_tensor("out", [ROWS_PER_CORE, D_IN], BF16, kind="ExternalOutput")

    with tile.TileContext(nc) as tc:
        with (
            tc.tile_pool(name="persist", bufs=1) as pers,
            tc.tile_pool(name="dram", bufs=1, space="DRAM") as dram,
        ):
            # ---- persistent SBUF preloads (contiguous, host-transposed) ----
            # weights preload from the (otherwise idle) scalar queue so the
            # sync queue starts issuing x chunks immediately
            wq_sb = pers.tile([128, KT_TILES, HPC * HD], BF16)
            wkv_sb = pers.tile([128, KT_TILES, 2 * HD], BF16)
            nc.scalar.dma_start(wq_sb[:, 0:8, :], wq[:, 0:8, :])
            nc.scalar.dma_start(wkv_sb[:], wkv[:])
            nc.scalar.dma_start(wq_sb[:, 8:16, :], wq[:, 8:16, :])

            cq_sb = pers.tile([128, SEQ], BF16)
            sq_sb = pers.tile([128, SEQ], BF16)
            ck_sb = pers.tile([64, SEQ], BF16)
            sk_sb = pers.tile([64, SEQ], BF16)
            mask_sb = pers.tile([128, 2, 128], BF16)

            def load_rope_tables():
                # issued on the sync queue AFTER block-0's x chunks: in-order
                # issue keeps the first matmuls' data ahead of these
                nc.sync.dma_start(ck_sb[:], coswk[:])
                nc.sync.dma_start(sk_sb[:], sinwk[:])
                nc.sync.dma_start(cq_sb[:], coswq[:])
                nc.sync.dma_start(sq_sb[:], sinwq[:])

            wo_sb = pers.tile([128, KT_TILES, D_IN], BF16)  # 8 MB
            sel_sb = pers.tile([2 * N_CORES, 2 * N_CORES, 128], F32R)

            ident = pers.tile([128, 128], F32)
            make_identity(nc, ident[:])
            eps_sb = pers.tile([128, 1], F32)
            nc.vector.memset(eps_sb[:], EPS)
            # block-diagonal ones (64x64 blocks), bf16, built on-chip: sums
            # the two packed heads separately in the psn matmul. A DMA'd
            # F32R version of this clogged the SDMA engines with thousands
            # of tiny descriptors (3.5us DGE) and wedged the whole head.
            onesblk = pers.tile([128, 128], BF16)
            nc.vector.memset(onesblk[:, :], 0.0)
            nc.vector.memset(onesblk[0:64, 0:64], 1.0)
            nc.vector.memset(onesblk[64:128, 64:128], 1.0)
            # causal 0/1 mask for the 128-wide diagonal strip (dup'd per u),
            # built on gpsimd (same reason: its 3-D DMA cost 7.3us of DGE)
            nc.gpsimd.memset(mask_sb[:], 1.0)
            nc.gpsimd.affine_select(
                out=mask_sb[:],
                in_=mask_sb[:],
                compare_op=mybir.AluOpType.is_ge,
                fill=0.0,
                base=0,
                channel_multiplier=-1,
                pattern=[[0, 2], [1, 128]],
            )

            # pair-stacked q (pair p holds heads 2p/2p+1 in partition halves)
            qt = [pers.tile([128, 2, QB], BF16, name=f"qt{j}") for j in range(NQB)]
            # k duplicated into both partition halves for row-group packing
            kt = [pers.tile([128, QB], BF16, name=f"kt{j}") for j in range(NQB)]
            vaug = [pers.tile([128, 4, HD + 1], BF16, name=f"va{j}") for j in range(NQB)]

            a2a_in = [
                dram.tile([N_CORES, 2, HD + 1, ROWS_PER_CORE], BF16, name=f"a2ai{p}")
                for p in range(2)
            ]
            a2a_out = [
                dram.tile([N_CORES, 2, HD + 1, ROWS_PER_CORE], BF16, name=f"a2ao{p}")
                for p in range(2)
            ]
            cc_warm_in = dram.tile([N_CORES, 4], F32, name="ccwi")
            cc_warm_out = dram.tile([N_CORES, 4], F32, name="ccwo")

            # ============ fused projections + attention ====================
            with (
                tc.tile_pool(name="xres", bufs=2) as xp,
                tc.tile_pool(name="acc", bufs=2, space="PSUM") as psA,
                tc.tile_pool(name="sc", bufs=2, space="PSUM") as psB,
                tc.tile_pool(name="pv", bufs=2, space="PSUM") as psPV,
                tc.tile_pool(name="work", bufs=2) as t1,
                tc.tile_pool(name="ptp", bufs=3) as ptp,
            ):
                xblk = {}

                def load_x(j, gate=None):
                    """SBUF-resident x for block j: [128, 16, 512] bf16, one
                    2MB load in 2 contiguous chunks (the host pre-arranges x
                    block-major so each chunk is an 8KB-per-partition run --
                    the k-tile-major layout fragmented into ~1KB descriptors
                    and clogged the SDMA engines for ~50us). `gate` (an AP
                    written by earlier compute) head-blocks the sync queue
                    via a tiny store, so these descriptors don't enter the
                    SDMA round-robin until the gate value exists."""
                    if gate is not None:
                        nc.sync.dma_start(gate_dram[:], gate)
                    xb = xp.tile([128, KT_TILES, QB], BF16, tag="x", name=f"x{j}")
                    for kk in range(0, KT_TILES, 8):
                        nc.sync.dma_start(
                            xb[:, kk : kk + 8, :], xT[:, j, kk : kk + 8, :]
                        )
                    xblk[j] = xb

                def norm_rope(j, raw_psum, idx):
                    """Evict + rmsnorm + rope one accumulator. idx 0/1 = q
                    pairs, idx 2 = kv. Generator (yields mid-chain)."""
                    sl = slice(QB * j, QB * j + QB)
                    is_kv = idx == 2
                    rows = slice(0, 64) if is_kv else slice(0, 128)
                    rawsb = t1.tile([128, QB], BF16, tag="rawsb")
                    nc.vector.tensor_copy(rawsb[:], raw_psum[:])
                    sq = t1.tile([128, QB], BF16, tag="sq")
                    nc.vector.tensor_mul(sq[:], rawsb[:], rawsb[:])
                    psn = psB.tile([128, 2, QB], F32, tag="sc", name=f"psn{idx}_{j}")
                    nc.tensor.matmul(psn[:, 0, :], onesblk[:], sq[:], start=True, stop=True)
                    lnv = t1.tile([128, QB], F32, tag="lnv", bufs=1)
                    nc.scalar.activation(
                        out=lnv[rows, :], in_=psn[rows, 0, :],
                        func=AF.Ln, bias=eps_sb[rows, :], scale=1.0 / HD,
                    )
                    rcp = t1.tile([128, QB], BF16, tag="rcp", bufs=1)
                    nc.scalar.activation(
                        out=rcp[rows, :], in_=lnv[rows, :], func=AF.Exp, scale=-0.5,
                    )
                    yield
                    tn = t1.tile([128, QB], BF16, tag="tn")
                    nc.vector.tensor_mul(tn[rows, :], rawsb[rows, :], rcp[rows, :])
                    rot = t1.tile([128, QB], BF16, tag="rot")
                    nh = 1 if is_kv else 2
                    for b in range(nh):
                        o = 64 * b
                        nc.vector.tensor_copy(rot[o : o + 32, :], tn[o + 32 : o + 64, :])
                        nc.vector.tensor_copy(rot[o + 32 : o + 64, :], tn[o : o + 32, :])
                    if is_kv:
                        tcs = t1.tile([64, QB], BF16, tag="tcs", bufs=1)
                        nc.vector.tensor_mul(tcs[:], tn[0:64, :], ck_sb[:, sl])
                        nc.vector.tensor_mul(rot[0:64, :], rot[0:64, :], sk_sb[:, sl])
                        nc.vector.tensor_add(kt[j][0:64, :], tcs[:], rot[0:64, :])
                        nc.vector.tensor_copy(kt[j][64:128, :], kt[j][0:64, :])
                        vt = t1.tile([64, QB], F32, tag="vt", bufs=1)
                        nc.vector.tensor_copy(vt[:], rawsb[64:128, :])
                        for d in range(4):
                            psv = psB.tile([128, 2, QB], F32, tag="sc", name=f"psv{j}_{d}")
                            nc.tensor.transpose(
                                psv[:, 0, 0:64],
                                vt[:, 128 * d : 128 * d + 128],
                                ident[0:64, 0:64],
                            )
                            nc.vector.tensor_copy(vaug[j][:, d, 0:HD], psv[:, 0, 0:64])
                            nc.vector.memset(vaug[j][:, d, HD : HD + 1], 1.0)
                            if d == 1:
                                yield
                    else:
                        tc2 = t1.tile([128, QB], BF16, tag="tc2")
                        nc.vector.tensor_mul(tc2[:], tn[:], cq_sb[:, sl])
                        nc.vector.tensor_mul(rot[:], rot[:], sq_sb[:, sl])
                        nc.vector.tensor_add(qt[j][:, idx, :], tc2[:], rot[:])
                    yield

                def emit_proj(j):
                    """Two-pass projection: Q (2 banks) then KV (1 bank).
                    Both passes read the SBUF-resident x block (no HBM)."""
                    xb = xblk.pop(j)
                    accq = [
                        psA.tile([128, QB], F32, tag="acc", name=f"accq{i}_{j}")
                        for i in range(2)
                    ]
                    for k in range(KT_TILES):
                        st = k == 0
                        sp = k == KT_TILES - 1
                        nc.tensor.matmul(accq[0][:], wq_sb[:, k, 0:128], xb[:, k, :], start=st, stop=sp)
                        nc.tensor.matmul(accq[1][:], wq_sb[:, k, 128:256], xb[:, k, :], start=st, stop=sp)
                        if k % 2 == 1:
                            yield
                    yield from norm_rope(j, accq[0], 0)
                    # block j+1's x load: its WAR on the xres pool slot
                    # (block j-1's matmuls) is what actually delays it --
                    # Tile schedules queues by dependency, not emission order
                    if j + 1 < NQB:
                        load_x(j + 1)
                    yield from norm_rope(j, accq[1], 1)
                    acckv = psA.tile([128, QB], F32, tag="acc", name=f"acckv_{j}")
                    for k in range(KT_TILES):
                        st = k == 0
                        sp = k == KT_TILES - 1
                        nc.tensor.matmul(acckv[:], wkv_sb[:, k, :], xb[:, k, :], start=st, stop=sp)
                        if k % 4 == 3:
                            yield
                    yield from norm_rope(j, acckv, 2)

                def emit_att_pair(j, p):
                    """Attention for block j, head pair p: scores for heads
                    2p/2p+1 run concurrently in PE row groups 0/64. Yields
                    after each unit."""
                    pvs = [
                        psPV.tile([128, QB], F32, tag="pv", name=f"pv{j}_{p}_{u}")
                        for u in range(2)
                    ]
                    ntile = 4 * j + 4
                    for t in range(ntile):
                        jj, d = t // 4, t % 4
                        diag = jj == j
                        n0 = 128 * d if diag else 0
                        w = QB - n0
                        sc = psB.tile([128, 2, QB], F32, tag="sc", name=f"sc{j}_{p}_{t}")
                        for u in range(2):
                            nc.tensor.matmul(
                                sc[:, u, 0:w],
                                kt[jj][64 * u : 64 * u + 64, 128 * d : 128 * d + 128],
                                qt[j][64 * u : 64 * u + 64, p, n0:QB],
                                start=True, stop=True,
                            )
                        pt = ptp.tile([128, 2, QB], BF16, tag="pt")
                        nc.scalar.activation(
                            out=pt[:, :, 0:w], in_=sc[:, :, 0:w],
                            func=AF.Exp, scale=0.125,
                        )
                        if diag:
                            # causal mask inside the 128-wide diagonal strip:
                            # 0/1 multiply on the bf16 exp output (2x DVE)
                            nc.vector.tensor_mul(
                                pt[:, :, 0:128], pt[:, :, 0:128], mask_sb[:]
                            )
                        for u in range(2):
                            nc.tensor.matmul(
                                pvs[u][0:65, n0:QB],
                                vaug[jj][:, d, :],
                                pt[:, u, 0:w],
                                start=(t == 0), stop=(t == ntile - 1),
                            )
                        if t % 2 == 1 or diag:
                            yield
                    for u in range(2):
                        att = t1.tile([65, QB], BF16, tag="att", bufs=8)
                        nc.vector.tensor_copy(att[:], pvs[u][0:65, :])
                        # late stores ride the gpsimd queue: its FIFO
                        # order (stores -> chunk trigger -> stores ->
                        # trigger) gives the collectives exact alias-free
                        # deps (sync-lane sem rotation added ~20us of false
                        # wait to the chunk-0 doorbell). Early pair-0 stores
                        # stay on sync: the gpsimd queue is blocked by the
                        # warmup collective's completion wait until ~90us.
                        # pair-0 stores (incl. block 3) on sync: its sem
                        # lanes are clean now that pair-1 left them, so the
                        # chunk-0 doorbell fires right at att(3,0)'s end
                        # instead of ~25us later behind reordered pair-1
                        # stores on the gpsimd FIFO
                        eng = nc.gpsimd if p == 1 else nc.sync
                        for s in range(2):
                            shard = 2 * j + s
                            cs = slice(ROWS_PER_CORE * s, ROWS_PER_CORE * (s + 1))
                            eng.dma_start(a2a_in[p][shard, u, :, :], att[:, cs])
                    yield

                def chain(gens):
                    for g in gens:
                        yield from g

                def drive(gen):
                    for _ in gen:
                        pass

                def interleave(att_gen, proj_gen, att_per_proj=1):
                    att_done = proj_done = False
                    while not (att_done and proj_done):
                        for _ in range(att_per_proj):
                            if not att_done:
                                att_done = next(att_gen, "END") == "END"
                        if not proj_done:
                            proj_done = next(proj_gen, "END") == "END"

                # warmup collective, fired immediately: prepays the
                # ~25-40us first-collective ncfw setup during the early
                # compute phases (its overhead provably doesn't stall the
                # other engines), so the real chunk-0 AllToAll is cheap.
                nc.gpsimd.collective_compute(
                    "AllToAll",
                    mybir.AluOpType.bypass,
                    replica_groups=[list(range(N_CORES))],
                    ins=[cc_warm_in[:].opt()],
                    outs=[cc_warm_out[:].opt()],
                )
                load_x(0)
                load_rope_tables()
                drive(emit_proj(0))
                nc.scalar.dma_start(sel_sb[:], sel16[:])
                # pair-major attention: all blocks' pair 0 first, so the
                # pair-0 AllToAll fires as early as possible
                interleave(emit_att_pair(0, 0), emit_proj(1), att_per_proj=1)
                interleave(emit_att_pair(1, 0), emit_proj(2), att_per_proj=1)
                # wo preload: Tile hoists dependency-free DMAs to the
                # front of the queue, so a queue-position "gate" does
                # nothing. Instead create a real WAR: a tiny DVE copy of
                # qt[2] into wo_sb's first bytes forces the 8MB load to
                # wait until block-2's q rope exists, keeping it out of the
                # head DMA round-robin.
                nc.vector.tensor_copy(wo_sb[0:1, 0, 0:8], qt[2][0:1, 0, 0:8])
                nc.scalar.dma_start(wo_sb[:], wo[:])
                interleave(emit_att_pair(2, 0), emit_proj(3), att_per_proj=1)
                drive(emit_att_pair(3, 0))

                # ---- reshard chunk 0 + pair-1 attention + out-projection ----
                R = ROWS_PER_CORE
                dsb_raw = [
                    t1.tile([2 * N_CORES, R], BF16, tag=f"denraw{i}", name=f"denraw{i}", bufs=1)
                    for i in range(2)
                ]
                dsb_inv = [
                    t1.tile([2 * N_CORES, R], F32, tag=f"deninv{i}", name=f"deninv{i}", bufs=1)
                    for i in range(2)
                ]
                dsb = [
                    t1.tile([2 * N_CORES, R], F32R, tag=f"den{i}", name=f"den{i}", bufs=1)
                    for i in range(2)
                ]
                an_sb = pers.tile([128, 2 * N_CORES, R], BF16)

                def emit_chunk(p):
                    nc.gpsimd.collective_compute(
                        "AllToAll",
                        mybir.AluOpType.bypass,
                        replica_groups=[list(range(N_CORES))],
                        ins=[a2a_in[p][:].opt()],
                        outs=[a2a_out[p][:].opt()],
                    )

                def emit_den(half):
                    # on sync: the sync queue is otherwise idle in phase C
                    # (stores moved to gpsimd), so these chunk-gated loads
                    # head-block nothing
                    for u in range(2):
                        nc.sync.dma_start(
                            dsb_raw[half][8 * u : 8 * u + 8, :],
                            a2a_out[half][:, u, 64, :],
                        )

                def an_half(half, bc_pool, bc_tag):
                    """Generator: normalize the attnT slices for chunk
                    `half` (one unit per source core)."""
                    dcast = t1.tile([2 * N_CORES, R], F32, tag=f"dcast{half}",
                                    name=f"dcast{half}", bufs=1)
                    nc.vector.tensor_copy(dcast[:], dsb_raw[half][:, :])
                    nc.vector.reciprocal_approx_fast(
                        out=dsb_inv[half][:, :], in_=dcast[:]
                    )
                    nc.vector.tensor_copy(dsb[half][:, :], dsb_inv[half][:, :])
                    for g in range(N_CORES):
                        a_raw = t1.tile([128, R], BF16, tag="araw")
                        nc.sync.dma_start(
                            a_raw[0:64, :], a2a_out[half][g, 0, 0:64, :]
                        )
                        nc.sync.dma_start(
                            a_raw[64:128, :], a2a_out[half][g, 1, 0:64, :]
                        )
                        bc = bc_pool.tile(
                            [128, QB], F32, tag=bc_tag, name=f"bc{half}_{g}"
                        )
                        nc.tensor.matmul(
                            bc[:, 0:R],
                            sel_sb[:, 2 * g + half, :],
                            dsb[half][:, :],
                            start=True, stop=True,
                        )
                        nc.vector.tensor_mul(
                            an_sb[:, 2 * g + half, :], a_raw[:], bc[:, 0:R]
                        )
                        yield

                emit_chunk(0)
                emit_den(0)
                # pair-1 attention for blocks 0-2 runs while chunk 0
                # transfers; an0 (gated on chunk 0) only joins for the last
                # block so its DVE ops can't head-block the diag masks
                drive(chain([emit_att_pair(j, 1) for j in range(NQB - 1)]))
                an0 = an_half(0, psA, "acc")
                interleave(emit_att_pair(NQB - 1, 1), an0, att_per_proj=2)
                drive(an0)
                emit_chunk(1)
                emit_den(1)

                # out projection: nb 0-2 get six accumulators (acc, pv, and
                # both halves of one sc slot) so all their half-0 work runs
                # under the chunk-1 transfer; the an-half-1 bc matmuls use
                # the second sc slot; nb3 runs last in the acc slots.
                poA = [psA.tile([128, QB], F32, tag="acc", name=f"poA{q}") for q in range(2)]
                poB = [psPV.tile([128, QB], F32, tag="pv", name=f"poB{q}") for q in range(2)]
                poCt = psB.tile([128, 2, QB], F32, tag="sc", name="poC")
                po_aps = {
                    0: [poA[0][:], poA[1][:]],
                    1: [poB[0][:], poB[1][:]],
                    2: [poCt[:, 0, :], poCt[:, 1, :]],
                }

                def po_mm(nb, q, gh, first, last):
                    nc.tensor.matmul(
                        po_aps[nb][q],
                        an_sb[:, gh, 128 * q : 128 * q + 128],
                        wo_sb[:, gh, 512 * nb : 512 * nb + 512],
                        start=first, stop=last,
                    )

                for g in range(N_CORES):
                    for nb in range(3):
                        for q in range(2):
                            po_mm(nb, q, 2 * g, g == 0, False)
                an1 = an_half(1, psB, "sc")
                for g in range(N_CORES):
                    next(an1, None)
                    for nb in range(3):
                        for q in range(2):
                            po_mm(nb, q, 2 * g + 1, False, g == N_CORES - 1)
                drive(an1)

                def po_evict(nb):
                    for q in range(2):
                        osb = t1.tile([128, QB], BF16, tag="osb")
                        nc.vector.tensor_copy(osb[:], po_aps[nb][q])
                        nc.sync.dma_start(
                            out[128 * q : 128 * q + 128, 512 * nb : 512 * nb + 512],
                            osb[:],
                        )

                po_evict(0)
                po_last = [psA.tile([128, QB], F32, tag="acc", name=f"poD{q}") for q in range(2)]
                po_aps[3] = [po_last[0][:], po_last[1][:]]
                for half in range(2):
                    for g in range(N_CORES):
                        gh = 2 * g + half
                        for q in range(2):
                            po_mm(3, q, gh, half == 0 and g == 0,
                                  half == 1 and g == N_CORES - 1)
                po_evict(1)
                po_evict(2)
                po_evict(3)

    nc.compile()
    return nc


_NC_CACHE = None


def _get_nc():
    global _NC_CACHE
    if _NC_CACHE is None:
        _NC_CACHE = _build()
    return _NC_CACHE


def _to_ktile_layout(w):
    m = w.shape[1]
    return np.ascontiguousarray(w.reshape(KT_TILES, 128, m).transpose(1, 0, 2))


def _make_in_maps(x, cos, sin, wq, wk, wv, wo, q_norm_w, k_norm_w):
    x = np.asarray(x, dtype=np.float32)
    cos = np.asarray(cos, dtype=np.float32)
    sin = np.asarray(sin, dtype=np.float32)
    wq = np.asarray(wq, dtype=np.float32)
    wk = np.asarray(wk, dtype=np.float32)
    wv = np.asarray(wv, dtype=np.float32)
    wo = np.asarray(wo, dtype=np.float32)
    qw = np.asarray(q_norm_w, dtype=np.float32)
    kw = np.asarray(k_norm_w, dtype=np.float32)

    xk = _to_ktile_layout(np.ascontiguousarray(x[0].T))  # [128, 16, 2048]
    xT = np.ascontiguousarray(
        xk.reshape(128, KT_TILES, NQB, QB).transpose(0, 2, 1, 3)
    ).astype(BF16_NP)  # [128, 4, 16, 512] block-major
    wo_b = _to_ktile_layout(wo).astype(BF16_NP)

    cosT = cos.T  # [64, SEQ]
    sinT = sin.T
    sgn = np.where(np.arange(64) < 32, -1.0, 1.0).astype(np.float32)
    wrot_q = qw[(np.arange(64) + 32) % 64]
    wrot_k = kw[(np.arange(64) + 32) % 64]
    cq1 = cosT * qw[:, None]
    sq1 = sinT * (sgn * wrot_q)[:, None]
    coswq = np.ascontiguousarray(np.vstack([cq1, cq1])).astype(BF16_NP)
    sinwq = np.ascontiguousarray(np.vstack([sq1, sq1])).astype(BF16_NP)
    coswk = np.ascontiguousarray(cosT * kw[:, None]).astype(BF16_NP)
    sinwk = np.ascontiguousarray(sinT * (sgn * wrot_k)[:, None]).astype(BF16_NP)

    sel16 = np.zeros((2 * N_CORES, 2 * N_CORES, 128), np.float32)
    for g in range(N_CORES):
        for half in range(2):
            for m in range(128):
                sel16[8 * (m // 64) + g, 2 * g + half, m] = 1.0

    in_maps = []
    for c in range(N_CORES):
        wq_c = _to_ktile_layout(
            np.ascontiguousarray(wq[:, 256 * c : 256 * c + 256])
        ).astype(BF16_NP)
        wkv_c = _to_ktile_layout(
            np.ascontiguousarray(
                np.concatenate(
                    [wk[:, 64 * c : 64 * c + 64], wv[:, 64 * c : 64 * c + 64]],
                    axis=1,
                )
            )
        ).astype(BF16_NP)
        in_maps.append(
            {
                "xT": xT,
                "wq": wq_c,
                "wkv": wkv_c,
                "wo": wo_b,
                "coswq": coswq,
                "sinwq": sinwq,
                "coswk": coswk,
                "sinwk": sinwk,
                "sel16": sel16,
            }
        )
    return in_maps


def kernel(x, cos, sin, wq, wk, wv, wo, q_norm_w, k_norm_w):
    in_maps = _make_in_maps(x, cos, sin, wq, wk, wv, wo, q_norm_w, k_norm_w)
    nc = _get_nc()
    res = run_bass_kernel_spmd(nc, in_maps, core_ids=list(range(N_CORES)))
    rows = [res.results[c]["out"] for c in range(N_CORES)]
    full = np.concatenate(rows, axis=0)  # [SEQ, D_IN]
    return full.reshape(1, SEQ, D_IN).astype(np.float32)
